# revision 24
# baseline (speedup 1.0000x reference)
"""Trainium2 Bass kernel for nn_Decoder (DDSP-style decoder).

Pure data-parallel over batch (32 -> 4 per core x 8 cores). Per core the
oscillator rows (4 batch x 32 osc) fill the 128 partitions exactly.
Phase is synthesized in *turns* by block-affine fp32 matmuls, range-reduced
with magic-constant rounding + an accumulating -I matmul, evaluated with the
ScalarE Sin LUT (accurate on [-pi, pi]), multiplied by the matmul-synthesized
amplitude envelope, and reduced over oscillators with a selector matmul.
The noise branch runs as real-DFT basis matmuls (no FFT instructions) at
float32r (full-rate PE).

Program structure is two "eras" split by a single activation-table switch:
era 1 = frontend + residual stacks + osc-grid activations (Sigmoid table),
era 2 = per-band scan machinery + noise branch + harmonic chunks (Sin table),
with band k's chunks interleaved right after band k's noise work.
"""
import numpy as np
import sys

sys.path.insert(0, "/opt/trn_rl_repo")

from concourse import bacc, mybir  # noqa: E402
from concourse.tile import TileContext  # noqa: E402
from concourse.bass_utils import run_bass_kernel_spmd  # noqa: E402

F32 = mybir.dt.float32
F32R = mybir.dt.float32r
BF16 = mybir.dt.bfloat16
ALU = mybir.AluOpType
BAND_SIZES = [512, 1024, 2048, 4096, 8192, 16384]
ADJUST = {512: 0.05, 1024: 0.03, 2048: 0.05, 4096: 0.25, 8192: 1.0, 16384: 20.0}
B, C, N_OSC, NNF = 32, 64, 32, 64
NCORE = 8
BL = B // NCORE          # 4 local batch
FR = BL * NNF            # 256 noise frames per core
MAGIC = float(1.5 * 2 ** 23)
TWO_PI = float(2 * np.pi)
TOTAL = 2 * sum(BAND_SIZES)  # 64512
USE_F32R_NOISE = True

_nc_cache = {}

W64_ORDER = ([f'up{i}d{dd}' for i in range(3) for dd in range(3)]
             + [f'find{dd}' for dd in range(3)]
             + [w for k in range(6) for w in
                [f't{k}0', f't{k}1', f't{k}2', f't{k}3', f'bf{k}', f'nup{k}']]
             + ['ident'])
W64_IDX = {n: i for i, n in enumerate(W64_ORDER)}
B64_ORDER = ([f'up{i}' for i in range(3)] + ['fin']
             + [b for k in range(6) for b in
                [f't{k}0', f't{k}1', f't{k}2', f't{k}3', f'bf{k}', f'nup{k}']])
B64_IDX = {n: i for i, n in enumerate(B64_ORDER)}
W3264_IDX = {}
for k in range(6):
    W3264_IDX[f'amp{k}'] = 2 * k
    W3264_IDX[f'frq{k}'] = 2 * k + 1

HARM_OFF = {}
NZ_OFF = {}
_off = 0
for _k, _bs in enumerate(BAND_SIZES):
    HARM_OFF[_k] = _off
    NZ_OFF[_k] = _off + _bs
    _off += 2 * _bs


# ---------------------------------------------------------------- host consts
def _interp_vecs(u):
    r = np.arange(u)
    f = (r + 0.5) / u - 0.5
    gm = np.where(r < u // 2, -f, 0.0)
    g0 = np.where(r < u // 2, 1 + f, 1 - f)
    gp = np.where(r >= u // 2, f, 0.0)
    return gm, g0, gp


def _build_U(n):
    eye = np.eye(n)
    spec = np.fft.rfft(eye, axis=-1)
    spec = np.pad(spec, ((0, 0), (0, n + 1 - spec.shape[-1])))
    return np.fft.irfft(spec, n=2 * n, axis=-1) * 2  # (n, 2n)


def _layout(ents):
    off = {}
    o = 0
    for name, r, cdim in ents:
        off[name] = (r, o, cdim)
        o += cdim
    return off, o


def _mega_ents():
    # f32 constants.  head0 | head1 | pb tail (split points for load priority)
    ents = [('wlin', C + 1, 4 * C), ('ubd4', BL * 4, BL * 8), ('ubd8', BL * 8, BL * 16),
            ('ubd16', BL * 16, BL * 32), ('bias64', C, len(B64_ORDER)),
            ('bias32', N_OSC, 12), ('ident128', 128, 128),
            ('w64', C, len(W64_ORDER) * C), ('w3264', C, 12 * N_OSC)]
    for k in (5, 0, 1, 2, 3, 4):
        ents.append((f'pb{k}', 128, 512))
    return ents


def _megar_ents():
    # float32r constants (noise branch DFT + coeff weights)
    ents = []
    for k in (5, 0, 1, 2, 3, 4):
        bs = BAND_SIZES[k]
        spf = bs // NNF
        nc_ = spf // 2 + 1
        ents.append((f'wc{k}', C + 1, nc_))
        if spf <= 128:
            ents.append((f'ft{k}', spf, spf))
            ents.append((f'ir{k}', spf, spf))
        else:
            for hh in range(2):
                ents.append((f'ft{k}_{hh}', 128, spf))
                ents.append((f'ir{k}_{hh}', 128, spf))
    return ents


def _megab_ents():
    # bf16 constants
    ents = [('negI', 128, 128), ('selstrip', 128, 256)]
    for k in (5, 0, 1, 2, 3, 4):
        ents.append((f'ab{k}', 128, 512))
    return ents


MEGA_OFF, MEGA_COLS = _layout(_mega_ents())
MEGAR_OFF, MEGAR_COLS = _layout(_megar_ents())
MEGAB_OFF, MEGAB_COLS = _layout(_megab_ents())
# load-priority split points in mega (cols)
MEGA_HEAD0 = MEGA_OFF['w64'][1]          # wlin/ubd/biases/ident128
MEGA_HEAD1 = MEGA_OFF['pb5'][1]          # w64/w3264
MEGA_HEAD2 = MEGA_OFF['pb0'][1]          # pb5
MEGAB_HEAD = MEGAB_OFF['ab0'][1]         # negI/selstrip/ab5
MEGAR_HEAD = MEGAR_OFF['wc0'][1]         # band-5 wc/ft/ir


def _band_bases(bs):
    u = bs // 32
    Bc = 512 // u
    gm, g0, gp = _interp_vecs(u)
    Gm, G0, Gp = np.cumsum(gm), np.cumsum(g0), np.cumsum(gp)
    pb = np.zeros((128, 512))
    ab = np.zeros((128, 512))
    inv = 1.0 / ADJUST[bs]
    sc = 8.0 / u   # compensates the u/8 folded into the incg grid scale
    for qq in range(Bc):
        cols = slice(qq * u, (qq + 1) * u)
        pb[0 * Bc + qq, cols] = Gm * sc
        pb[1 * Bc + qq, cols] = G0 * sc
        pb[2 * Bc + qq, cols] = Gp * sc
        pb[3 * Bc + qq, cols] = 1.0
        ab[0 * Bc + qq, cols] = gm * inv
        ab[1 * Bc + qq, cols] = g0 * inv
        ab[2 * Bc + qq, cols] = gp * inv
    return pb.astype(np.float32), ab.astype(np.float32)


def _band_fir(bs):
    spf = bs // NNF
    nc_ = spf // 2 + 1
    t = np.arange(spf)
    j_re = np.arange(nc_)
    j_im = np.arange(1, nc_ - 1)
    FT = np.concatenate([np.cos(2 * np.pi * np.outer(t, j_re) / spf),
                         -np.sin(2 * np.pi * np.outer(t, j_im) / spf)], axis=1)
    w = np.full(nc_, 2.0)
    w[0] = 1.0
    w[-1] = 1.0
    IR = np.concatenate([
        (w[:, None] * np.cos(2 * np.pi * np.outer(j_re, t) / spf)) / spf,
        (-2.0 * np.sin(2 * np.pi * np.outer(j_im, t) / spf)) / spf,
    ], axis=0) / ADJUST[bs]
    return FT.astype(np.float32), IR.astype(np.float32)


def _build_shared(inp):
    import ml_dtypes
    c = {}
    wl = np.zeros((4, C + 1, C), np.float32)
    for t in range(4):
        wl[t, :C] = inp['up_lin_w'][:, t::4]
        wl[t, C] = inp['up_lin_b'][t::4]
    c['wlin'] = wl.transpose(1, 0, 2).reshape(C + 1, 4 * C)   # (65, 256), block t
    for n in (4, 8, 16):
        U = _build_U(n)
        ub = np.zeros((BL * n, BL * 2 * n), np.float32)
        for b in range(BL):
            ub[b * n:(b + 1) * n, b * 2 * n:(b + 1) * 2 * n] = U
        c[f'ubd{n}'] = ub

    w64 = np.zeros((C, len(W64_ORDER) * C), np.float32)

    def put64(name, m):
        i = W64_IDX[name]
        w64[:, i * C:(i + 1) * C] = m

    for i in range(3):
        for dd in range(3):
            put64(f'up{i}d{dd}', inp['up_conv_w'][i, :, :, dd].T)
    for dd in range(3):
        put64(f'find{dd}', inp['up_final_w'][:, :, dd].T)
    for k in range(6):
        for j in range(4):
            put64(f't{k}{j}', inp['t_w'][k, j].T)
        put64(f'bf{k}', inp['band_final_w'][k].T)
        put64(f'nup{k}', inp['noise_up_w'][k].T)
    put64('ident', np.eye(C))
    c['w64'] = w64

    w32 = np.zeros((C, 12 * N_OSC), np.float32)
    for k in range(6):
        w32[:, W3264_IDX[f'amp{k}'] * N_OSC:(W3264_IDX[f'amp{k}'] + 1) * N_OSC] = inp['osc_amp_w'][k].T
        w32[:, W3264_IDX[f'frq{k}'] * N_OSC:(W3264_IDX[f'frq{k}'] + 1) * N_OSC] = inp['osc_freq_w'][k].T
    c['w3264'] = w32

    b64 = np.zeros((C, len(B64_ORDER)), np.float32)
    for i in range(3):
        b64[:, B64_IDX[f'up{i}']] = inp['up_conv_b'][i]
    b64[:, B64_IDX['fin']] = inp['up_final_b']
    for k in range(6):
        for j in range(4):
            b64[:, B64_IDX[f't{k}{j}']] = inp['t_b'][k, j]
        b64[:, B64_IDX[f'bf{k}']] = inp['band_final_b'][k]
        b64[:, B64_IDX[f'nup{k}']] = inp['noise_up_b'][k]
    c['bias64'] = b64

    b32 = np.zeros((N_OSC, 12), np.float32)
    for k in range(6):
        b32[:, W3264_IDX[f'amp{k}']] = inp['osc_amp_b'][k]
        b32[:, W3264_IDX[f'frq{k}']] = inp['osc_freq_b'][k]
    c['bias32'] = b32
    c['ident128'] = np.eye(128, dtype=np.float32)

    cr = {}
    cb = {'negI': (-np.eye(128)).astype(np.float32)}
    sel = np.zeros((128, 256), np.float32)
    for b in range(BL):
        sel[b * N_OSC:(b + 1) * N_OSC, 128 + b] = 1.0
    cb['selstrip'] = sel

    for k, bs in enumerate(BAND_SIZES):
        nc_ = bs // NNF // 2 + 1
        w = np.zeros((C + 1, nc_), np.float32)
        w[:C] = inp[f'noise_coeff_w_{k}'].T
        w[C] = inp[f'noise_coeff_b_{k}']
        if k == 0:
            w[:, 1:] = 0.0
        cr[f'wc{k}'] = w
        FT, IR = _band_fir(bs)
        if bs // NNF <= 128:
            cr[f'ft{k}'] = FT
            cr[f'ir{k}'] = IR
        else:
            cr[f'ft{k}_0'], cr[f'ft{k}_1'] = FT[0:128], FT[128:256]
            cr[f'ir{k}_0'], cr[f'ir{k}_1'] = IR[0:128], IR[128:256]
        pb, ab = _band_bases(bs)
        c[f'pb{k}'] = pb
        cb[f'ab{k}'] = ab

    mega = np.zeros((128, MEGA_COLS), np.float32)
    for name, (r, o, cd) in MEGA_OFF.items():
        mega[0:r, o:o + cd] = c[name]
    megar = np.zeros((128, MEGAR_COLS), np.float32)
    for name, (r, o, cd) in MEGAR_OFF.items():
        megar[0:r, o:o + cd] = cr[name]
    megab = np.zeros((128, MEGAB_COLS), np.float32)
    for name, (r, o, cd) in MEGAB_OFF.items():
        megab[0:r, o:o + cd] = cb[name]
    return {'mega': mega, 'megar': megar,
            'megab': megab.astype(ml_dtypes.bfloat16)}


# ---------------------------------------------------------------- bass build
def _build_nc():
    nc = bacc.Bacc('TRN2', num_devices=NCORE)
    AF = mybir.ActivationFunctionType

    d = {}
    d['xT'] = nc.dram_tensor("xT", [C + 1, BL], F32, kind="ExternalInput")
    d['mega'] = nc.dram_tensor("mega", [128, MEGA_COLS], F32, kind="ExternalInput")
    d['megar'] = nc.dram_tensor("megar", [128, MEGAR_COLS], F32R if USE_F32R_NOISE else F32, kind="ExternalInput")
    d['megab'] = nc.dram_tensor("megab", [128, MEGAB_COLS], BF16, kind="ExternalInput")
    for k, bs in enumerate(BAND_SIZES):
        spf = bs // NNF
        d[f'noise{k}'] = nc.dram_tensor(f"noise{k}", [FR, spf], F32, kind="ExternalInput")
    out_d = nc.dram_tensor("out", [BL, TOTAL], F32, kind="ExternalOutput")

    with TileContext(nc) as tc:
        with tc.tile_pool(name="const", bufs=1) as cp, \
             tc.tile_pool(name="work", bufs=1) as wp, \
             tc.tile_pool(name="hot", bufs=4) as hot, \
             tc.tile_pool(name="dram", bufs=1, space="DRAM") as dp, \
             tc.tile_pool(name="pp", bufs=3, space="PSUM") as pp, \
             tc.tile_pool(name="pa", bufs=2, space="PSUM") as pa, \
             tc.tile_pool(name="ph", bufs=3, space="PSUM") as ph:

            # ---------------- const loads (priority order; chunks-of-band-5
            # need pb5/ab5/negI/selstrip early, then band-5 noise consts)
            mega = cp.tile([128, MEGA_COLS], F32, tag="mega")
            nc.sync.dma_start(out=mega[:, 0:MEGA_HEAD0],
                              in_=d['mega'][:, 0:MEGA_HEAD0])
            xT = cp.tile([C + 1, BL], F32, tag="xT")
            nc.sync.dma_start(out=xT, in_=d['xT'][:, :])
            nc.sync.dma_start(out=mega[:, MEGA_HEAD0:MEGA_HEAD1],
                              in_=d['mega'][:, MEGA_HEAD0:MEGA_HEAD1])
            nc.sync.dma_start(out=mega[:, MEGA_HEAD1:MEGA_HEAD2],
                              in_=d['mega'][:, MEGA_HEAD1:MEGA_HEAD2])
            megab = cp.tile([128, MEGAB_COLS], BF16, tag="megab")
            nc.sync.dma_start(out=megab[:, 0:MEGAB_HEAD],
                              in_=d['megab'][:, 0:MEGAB_HEAD])
            megar = cp.tile([128, MEGAR_COLS], F32R if USE_F32R_NOISE else F32, tag="megar")
            nc.sync.dma_start(out=megar[:, 0:MEGAR_HEAD],
                              in_=d['megar'][:, 0:MEGAR_HEAD])
            nbt = {}
            for k in (5, 0, 1, 2, 3, 4):
                spf = BAND_SIZES[k] // NNF
                nb0 = cp.tile([128, spf], F32, tag=f"nb0_{k}")
                nb1 = cp.tile([128, spf], F32, tag=f"nb1_{k}")
                nc.sync.dma_start(out=nb0, in_=d[f'noise{k}'][0:128, :])
                nc.sync.dma_start(out=nb1, in_=d[f'noise{k}'][128:256, :])
                nbt[k] = (nb0, nb1)
            nc.sync.dma_start(out=megar[:, MEGAR_HEAD:MEGAR_COLS],
                              in_=d['megar'][:, MEGAR_HEAD:MEGAR_COLS])
            nc.sync.dma_start(out=megab[:, MEGAB_HEAD:MEGAB_COLS],
                              in_=d['megab'][:, MEGAB_HEAD:MEGAB_COLS])
            nc.sync.dma_start(out=mega[:, MEGA_HEAD2:MEGA_COLS],
                              in_=d['mega'][:, MEGA_HEAD2:MEGA_COLS])

            ct = {}
            for name, (r, o, cd) in MEGA_OFF.items():
                ct[name] = mega[0:r, o:o + cd]
            for name, (r, o, cd) in MEGAR_OFF.items():
                ct[name] = megar[0:r, o:o + cd]
            for name, (r, o, cd) in MEGAB_OFF.items():
                ct[name] = megab[0:r, o:o + cd]

            def w64s(name):
                i = W64_IDX[name]
                return ct['w64'][:, i * C:(i + 1) * C]

            def w32s(name):
                i = W3264_IDX[name]
                return ct['w3264'][:, i * N_OSC:(i + 1) * N_OSC]

            def b64s(name):
                return ct['bias64'][:, B64_IDX[name]:B64_IDX[name] + 1]

            def b32s(name):
                i = W3264_IDX[name]
                return ct['bias32'][:, i:i + 1]

            ident64 = w64s('ident')

            # ================ ERA 1: frontend + stacks + grid activations
            h = wp.tile([C, 16], F32, tag="h0")
            for t in range(4):
                pt = pp.tile([C, BL], F32, tag="pp")
                nc.tensor.matmul(out=pt, lhsT=ct['wlin'][:, t * C:(t + 1) * C], rhs=xT,
                                 start=True, stop=True)
                nc.vector.tensor_copy(out=h.rearrange("c (b t) -> c b t", t=4)[:, :, t], in_=pt)

            for i, n in enumerate((4, 8, 16)):
                pt1 = pp.tile([BL * n, C], F32, tag="pp")
                nc.tensor.transpose(out=pt1, in_=h, identity=ct['ident128'][0:C, 0:C])
                t1 = wp.tile([BL * n, C], F32, tag=f"fe_t1_{i}")
                nc.vector.tensor_copy(out=t1, in_=pt1)
                pt2 = pp.tile([BL * 2 * n, C], F32, tag="pp")
                nc.tensor.matmul(out=pt2, lhsT=ct[f'ubd{n}'], rhs=t1, start=True, stop=True)
                t2 = wp.tile([BL * 2 * n, C], F32, tag=f"fe_t2_{i}")
                nc.vector.tensor_copy(out=t2, in_=pt2)
                pt3 = pp.tile([C, BL * 2 * n], F32, tag="pp")
                nc.tensor.transpose(out=pt3, in_=t2,
                                    identity=ct['ident128'][0:BL * 2 * n, 0:BL * 2 * n])
                hu = wp.tile([C, BL * 2 * n], F32, tag=f"fe_hu_{i}")
                nc.vector.tensor_copy(out=hu, in_=pt3)
                m = 2 * n
                hu3 = hu.rearrange("c (b t) -> c b t", b=BL)
                pc = pp.tile([C, BL, m], F32, tag="pp")
                nc.tensor.matmul(out=pc[:, :, :], lhsT=w64s(f'up{i}d1'), rhs=hu3[:, :, :],
                                 start=True, stop=False)
                nc.tensor.matmul(out=pc[:, :, 1:m], lhsT=w64s(f'up{i}d0'), rhs=hu3[:, :, 0:m - 1],
                                 start=False, stop=False)
                nc.tensor.matmul(out=pc[:, :, 0:m - 1], lhsT=w64s(f'up{i}d2'), rhs=hu3[:, :, 1:m],
                                 start=False, stop=True)
                h = wp.tile([C, BL * m], F32, tag=f"fe_h_{i}")
                nc.scalar.activation(out=h.rearrange("c (b t) -> c b t", b=BL), in_=pc,
                                     func=AF.Prelu, bias=b64s(f'up{i}'), scale=1.0, alpha=0.2)

            h3 = h.rearrange("c (b t) -> c b t", b=BL)
            pf = pp.tile([C, BL, 32], F32, tag="pp")
            nc.tensor.matmul(out=pf[:, :, :], lhsT=w64s('find1'), rhs=h3[:, :, :],
                             start=True, stop=False)
            nc.tensor.matmul(out=pf[:, :, 1:32], lhsT=w64s('find0'), rhs=h3[:, :, 0:31],
                             start=False, stop=False)
            nc.tensor.matmul(out=pf[:, :, 0:31], lhsT=w64s('find2'), rhs=h3[:, :, 1:32],
                             start=False, stop=True)
            hfin = cp.tile([C, 128], F32, tag="hfin")
            nc.scalar.activation(out=hfin.rearrange("c (b t) -> c b t", b=BL), in_=pf,
                                 func=AF.Identity, bias=b64s('fin'), scale=1.0)

            # Emission schedule (band 5 owns half the chunk work, so it
            # goes first and its 48us chunk block starts ~20us in; the other
            # bands' grid machinery + gather chains ride in hooks between
            # chunk groups, on engines the chunk pipeline leaves idle):
            #   frontend -> stacks+grids (5..0) -> machinery5+chain5
            #   -> noise (3 pipelined passes, all bands)
            #   -> chunks5 [hooks: A4|B4,A3|B3,A2|B2,A1] -> chunks4 [B1,A0|B0]
            #   -> chunks3..0
            GC = {'ipr': 0, 'incg': 128, 'inx': 256, 'Cf': 384,
                  'apr': 512, 'ampg': 640, 'anx': 768}
            zt32 = cp.tile([N_OSC, 32], F32, tag="zt32")
            nc.vector.memset(zt32, 0.0)
            ones_f32 = wp.tile([1, FR], F32, tag="ones_f32")
            nc.vector.memset(ones_f32, 1.0)
            # scan-reset mask: state = mask*state + Sr resets at batch starts
            maskt = cp.tile([N_OSC, 128], F32, tag="maskt")
            nc.vector.memset(maskt, 1.0)
            for b in range(BL):
                nc.vector.memset(maskt[:, 32 * b:32 * b + 1], 0.0)

            zfs, Gs, siggs = {}, {}, {}
            band_lhsT = {}
            pending = []
            osc_q = []

            zcur = {}

            def _stack_step(k, j):
                # one residual link; j-major emission keeps ScalarE streaming
                z = zcur.get(k, hfin)
                pz = pp.tile([C, 128], F32, tag="pp")
                nc.tensor.matmul(out=pz, lhsT=w64s(f't{k}{j}'), rhs=z, start=True, stop=False)
                nc.tensor.matmul(out=pz, lhsT=ident64, rhs=z, start=False, stop=True)
                z = wp.tile([C, 128], F32, tag=f"z_{k}_{j % 2}")
                nc.scalar.activation(out=z, in_=pz, func=AF.Prelu,
                                     bias=b64s(f't{k}{j}'), scale=1.0, alpha=0.2)
                zcur[k] = z

            def _stack_fin(k):
                pz = pp.tile([C, 128], F32, tag="pp")
                nc.tensor.matmul(out=pz, lhsT=w64s(f'bf{k}'), rhs=zcur[k], start=True, stop=True)
                zf = cp.tile([C, 128], F32, tag=f"zf_{k}")
                nc.scalar.activation(out=zf, in_=pz, func=AF.Identity, bias=b64s(f'bf{k}'),
                                     scale=1.0)
                zfs[k] = zf

            def _grids(k):
                zf = zfs[k]
                G = cp.tile([N_OSC, 896], F32, tag=f"G_{k}")
                Gs[k] = G
                pg = pa.tile([N_OSC, 128], F32, tag="pa")
                nc.tensor.matmul(out=pg, lhsT=w32s(f'amp{k}'), rhs=zf, start=True, stop=True)
                nc.scalar.activation(out=G[:, GC['ampg']:GC['ampg'] + 128], in_=pg,
                                     func=AF.Abs, bias=b32s(f'amp{k}'), scale=1.0)
                pg2 = pa.tile([N_OSC, 128], F32, tag="pa")
                nc.tensor.matmul(out=pg2, lhsT=w32s(f'frq{k}'), rhs=zf, start=True, stop=True)
                sigg = wp.tile([N_OSC, 128], F32, tag=f"sigg_{k}")
                nc.scalar.activation(out=sigg, in_=pg2, func=AF.Sigmoid, bias=b32s(f'frq{k}'),
                                     scale=1.0)
                siggs[k] = sigg

            def _machinery_a(k, eng=None):
                # incg2 = (freq/2)*(u/8) folded scale (pb G-rows host-scaled
                # by 8/u).  eng=nc.gpsimd rides in chunk-phase hooks (Pool is
                # idle there); band 5 runs pre-chunk on DVE only, avoiding
                # cross-engine sem hops on its critical path.
                eng = eng or nc.gpsimd
                bs = BAND_SIZES[k]
                u = bs // 32
                lf = 0.05 if bs == 512 else 0.01
                G = Gs[k]
                sigg = siggs[k]
                incg = G[:, GC['incg']:GC['incg'] + 128]
                nc.vector.tensor_scalar(out=incg, in0=sigg,
                                        scalar1=float((1.0 - lf) / 2.0 * u / 8.0),
                                        scalar2=float(lf / 2.0 * u / 8.0),
                                        op0=ALU.mult, op1=ALU.add)

                def shifted(src, pr, nx):
                    s3 = src.rearrange("o (b q) -> o b q", b=BL)
                    p3 = pr.rearrange("o (b q) -> o b q", b=BL)
                    n3 = nx.rearrange("o (b q) -> o b q", b=BL)
                    eng.tensor_copy(out=p3[:, :, 1:32], in_=s3[:, :, 0:31])
                    eng.tensor_copy(out=p3[:, :, 0:1], in_=s3[:, :, 0:1])
                    eng.tensor_copy(out=n3[:, :, 0:31], in_=s3[:, :, 1:32])
                    eng.tensor_copy(out=n3[:, :, 31:32], in_=s3[:, :, 31:32])

                ipr = G[:, GC['ipr']:GC['ipr'] + 128]
                inx = G[:, GC['inx']:GC['inx'] + 128]
                ampg = G[:, GC['ampg']:GC['ampg'] + 128]
                apr = G[:, GC['apr']:GC['apr'] + 128]
                anx = G[:, GC['anx']:GC['anx'] + 128]
                shifted(incg, ipr, inx)
                shifted(ampg, apr, anx)
                # S = ipr2 + 6*incg2 + inx2  (per-block increment, turns)
                t1 = wp.tile([N_OSC, 128], F32, tag=f"sg1_{k % 2}")
                eng.tensor_scalar(out=t1, in0=incg, scalar1=6.0, scalar2=None,
                                        op0=ALU.mult)
                t2 = wp.tile([N_OSC, 128], F32, tag=f"sg2_{k % 2}")
                eng.tensor_tensor(out=t2, in0=t1, in1=ipr, op=ALU.add)
                S = wp.tile([N_OSC, 128], F32, tag=f"sgS_{k % 2}")
                eng.tensor_tensor(out=S, in0=t2, in1=inx, op=ALU.add)
                Sn = wp.tile([N_OSC, 128], F32, tag=f"sgSn_{k % 2}")
                eng.tensor_scalar(out=Sn, in0=S, scalar1=MAGIC, scalar2=MAGIC,
                                        op0=ALU.add, op1=ALU.subtract)
                Sr = wp.tile([N_OSC, 128], F32, tag=f"sgSr_{k % 2}")
                eng.tensor_tensor(out=Sr, in0=S, in1=Sn, op=ALU.subtract)
                return Sr

            def _machinery_b(k, Sr, eng=None):
                eng = eng or nc.gpsimd
                bs = BAND_SIZES[k]
                Bc = 512 // (bs // 32)
                nch = bs // 512
                G = Gs[k]
                Pt = wp.tile([N_OSC, 128], F32, tag=f"sgPt_{k % 2}")
                nc.vector.tensor_tensor_scan(out=Pt, data0=maskt, data1=Sr,
                                             initial=0.0, op0=ALU.mult, op1=ALU.add)
                Ce = wp.tile([N_OSC, 128], F32, tag=f"sgCe_{k % 2}")
                Ce3 = Ce.rearrange("o (b q) -> o b q", b=BL)
                eng.tensor_copy(out=Ce3[:, :, 1:32],
                                in_=Pt.rearrange("o (b q) -> o b q", b=BL)[:, :, 0:31])
                eng.memset(Ce3[:, :, 0:1], 0.0)
                Cn = wp.tile([N_OSC, 128], F32, tag=f"sgCn_{k % 2}")
                eng.tensor_scalar(out=Cn, in0=Ce, scalar1=MAGIC, scalar2=MAGIC,
                                        op0=ALU.add, op1=ALU.subtract)
                eng.tensor_tensor(out=G[:, GC['Cf']:GC['Cf'] + 128],
                                  in0=Ce, in1=Cn, op=ALU.subtract)

                Tall = wp.tile([N_OSC, 896], F32, tag=f"Tall_{k}")
                nc.vector.transpose(out=Tall, in_=G)
                # DMA chain realizing the 4-dim permute (c,q,g,m)->(g,q,c,m).
                # SBUF partition dims cannot be split in DMA APs, so the
                # (c q)->(q c) row transpose happens DRAM->DRAM; when nch==1
                # or Bc==1 no row transpose is needed and Tall goes straight
                # to scrB (partition dim stays whole, cols split by g).
                lp = cp.tile([128, nch * 128], F32, tag=f"lp_{k}")
                la = cp.tile([128, nch * 128], BF16, tag=f"la_{k}")
                if Bc == 1:
                    # one hop: rows (r, g), SBUF partition dim stays outermost
                    scrB2 = dp.tile([7 * N_OSC, 128], F32, tag=f"scrB_{k}")
                    nc.sync.dma_start(
                        out=scrB2.rearrange("(r g) m -> r g m", g=7),
                        in_=Tall.rearrange("r (g m) -> r g m", m=128))
                    src_p = scrB2.rearrange("(r g) m -> g r m", g=7)
                    nc.sync.dma_start(
                        out=lp[0:4, :].rearrange("p (c m) -> p c m", m=128),
                        in_=src_p[0:4, :, :])
                    nc.gpsimd.dma_start(
                        out=la[0:3, :].rearrange("p (c m) -> p c m", m=128),
                        in_=src_p[4:7, :, :])
                else:
                    scrA = dp.tile([N_OSC, 896], F32, tag=f"scrA_{k}")
                    nc.sync.dma_start(out=scrA, in_=Tall)
                    if nch == 1:
                        scrT = scrA
                    else:
                        scrT = dp.tile([N_OSC, 896], F32, tag=f"scrT_{k}")
                        nc.sync.dma_start(
                            out=scrT.rearrange("(q c) m -> q c m", c=nch),
                            in_=scrA.rearrange("(c q) m -> q c m", q=Bc))
                    scrB = dp.tile([7 * N_OSC, 128], F32, tag=f"scrB_{k}")
                    nc.sync.dma_start(
                        out=scrB.rearrange("(g r) m -> g r m", g=7),
                        in_=scrT.rearrange("r (g m) -> g r m", m=128))
                    nc.sync.dma_start(
                        out=lp[0:4 * Bc, :].rearrange("p (c m) -> p c m", m=128),
                        in_=scrB[0:4 * N_OSC, :].rearrange("(p c) m -> p c m", c=nch))
                    nc.gpsimd.dma_start(
                        out=la[0:3 * Bc, :].rearrange("p (c m) -> p c m", m=128),
                        in_=scrB[4 * N_OSC:7 * N_OSC, :].rearrange("(p c) m -> p c m", c=nch))
                band_lhsT[k] = (lp, la)

            # noise branch in 3 pipelined passes (per-band tiles so bands
            # overlap; each pass's deps are ready when the queue reaches it)
            NDT = F32R if USE_F32R_NOISE else F32
            nTs_all, naug_all, pc_all, chat_all = {}, {}, {}, {}

            def _noise_i(k):
                spf = BAND_SIZES[k] // NNF
                zf = zfs[k]
                zf3 = zf.rearrange("c (b t) -> c b t", b=BL)
                zrep = zf3.unsqueeze(-1).broadcast_to([C, BL, 32, 2])
                pn = pp.tile([C, FR], F32, tag="pp")
                nc.tensor.matmul(out=pn, lhsT=w64s(f'nup{k}'), rhs=zrep, start=True, stop=True)
                naug = cp.tile([C + 1, FR], NDT, tag=f"naug_{k}")
                nc.vector.tensor_copy(out=naug[C:C + 1, :], in_=ones_f32)
                nc.scalar.activation(out=naug[0:C, :], in_=pn, func=AF.Prelu,
                                     bias=b64s(f'nup{k}'), scale=1.0, alpha=0.2)
                naug_all[k] = naug
                nb0, nb1 = nbt[k]
                if spf <= 128:
                    nT = wp.tile([spf, FR], NDT, tag=f"nT_{k}")
                    for hh, nb in enumerate((nb0, nb1)):
                        ptr = pp.tile([spf, 128], F32, tag="pp")
                        nc.tensor.transpose(out=ptr, in_=nb, identity=ct['ident128'])
                        nc.vector.tensor_copy(out=nT[:, 128 * hh:128 * (hh + 1)], in_=ptr)
                    nTs_all[k] = [nT]
                else:
                    nT0 = wp.tile([128, FR], NDT, tag="nT5_0")
                    nT1 = wp.tile([128, FR], NDT, tag="nT5_1")
                    for hh, nb in enumerate((nb0, nb1)):
                        for half, dst in ((0, nT0), (1, nT1)):
                            ptr = pp.tile([128, 128], F32, tag="pp")
                            nc.tensor.transpose(out=ptr, in_=nb[:, 128 * half:128 * (half + 1)],
                                                identity=ct['ident128'])
                            nc.vector.tensor_copy(out=dst[:, 128 * hh:128 * (hh + 1)], in_=ptr)
                    nTs_all[k] = [nT0, nT1]

            def _noise_ii(k):
                spf = BAND_SIZES[k] // NNF
                nc_ = spf // 2 + 1
                naug = naug_all[k]
                if nc_ <= 128:
                    pcA = pa.tile([nc_, FR], F32, tag="pa")
                    nc.tensor.matmul(out=pcA, lhsT=ct[f'wc{k}'], rhs=naug, start=True, stop=True)
                    pcB = None
                else:
                    pcA = pa.tile([128, FR], F32, tag="pa")
                    nc.tensor.matmul(out=pcA, lhsT=ct[f'wc{k}'][:, 0:128], rhs=naug,
                                     start=True, stop=True)
                    pcB = pp.tile([1, FR], F32, tag="pp")
                    nc.tensor.matmul(out=pcB, lhsT=ct[f'wc{k}'][:, 128:nc_], rhs=naug,
                                     start=True, stop=True)
                if spf <= 128:
                    chat = wp.tile([spf, FR], F32, tag=f"chat_{k}")
                    nc.scalar.copy(out=chat[0:nc_, :], in_=pcA)
                    if spf > nc_:
                        nc.scalar.dma_start(out=chat[nc_:spf, :], in_=chat[1:nc_ - 1, :])
                    chat_all[k] = [chat]
                else:
                    ch0 = wp.tile([128, FR], F32, tag="chat5_0")
                    nc.scalar.copy(out=ch0, in_=pcA)
                    ch1 = wp.tile([128, FR], F32, tag="chat5_1")
                    nc.scalar.copy(out=ch1, in_=pcA)
                    nc.scalar.copy(out=ch1[0:1, :], in_=pcB)
                    chat_all[k] = [ch0, ch1]

            def _noise_iii(k):
                bs = BAND_SIZES[k]
                spf = bs // NNF
                nTs = nTs_all[k]
                chats = chat_all[k]
                if spf <= 128:
                    psp = pp.tile([spf, FR], F32, tag="pp")
                    nc.tensor.matmul(out=psp, lhsT=ct[f'ft{k}'], rhs=nTs[0], start=True, stop=True)
                    sA = wp.tile([spf, FR], NDT, tag=f"sA_{k}")
                    nc.vector.tensor_tensor(out=sA, in0=chats[0], in1=psp, op=ALU.mult)
                    sAs = [sA]
                else:
                    sAs = []
                    for half in range(2):
                        psp = pp.tile([128, FR], F32, tag="pp")
                        nc.tensor.matmul(out=psp, lhsT=ct['ft5_0'][:, 128 * half:128 * (half + 1)],
                                         rhs=nTs[0], start=True, stop=False)
                        nc.tensor.matmul(out=psp, lhsT=ct['ft5_1'][:, 128 * half:128 * (half + 1)],
                                         rhs=nTs[1], start=False, stop=True)
                        sA = wp.tile([128, FR], NDT, tag=f"sA5_{half}")
                        nc.vector.tensor_tensor(out=sA, in0=chats[half], in1=psp, op=ALU.mult)
                        sAs.append(sA)
                for fg in range(2):
                    pnz = ph.tile([128, spf], F32, tag="ph")
                    if spf <= 128:
                        nc.tensor.matmul(out=pnz, lhsT=sAs[0][:, 128 * fg:128 * (fg + 1)],
                                         rhs=ct[f'ir{k}'], start=True, stop=True)
                    else:
                        nc.tensor.matmul(out=pnz, lhsT=sAs[0][:, 128 * fg:128 * (fg + 1)],
                                         rhs=ct['ir5_0'], start=True, stop=False)
                        nc.tensor.matmul(out=pnz, lhsT=sAs[1][:, 128 * fg:128 * (fg + 1)],
                                         rhs=ct['ir5_1'], start=False, stop=True)
                    nzs = wp.tile([128, spf], F32, tag=f"nzs_{k}")
                    nc.vector.tensor_copy(out=nzs, in_=pnz)
                    for j in range(2):
                        b_ = 2 * fg + j
                        nc.sync.dma_start(
                            out=out_d[b_:b_ + 1, NZ_OFF[k]:NZ_OFF[k] + bs]
                            .rearrange("o (f t) -> (o f) t", t=spf),
                            in_=nzs[NNF * j:NNF * (j + 1), :])

            def _emit_osc():
                while osc_q:
                    osc_q.pop(0)()

            def _flush_harm():
                for (hpt_, gs_, gstart_, k_) in pending:
                    hsb = wp.tile([4 * gs_, 512], F32, tag="hsb")
                    nc.scalar.copy(out=hsb, in_=hpt_[0:4 * gs_, :])
                    for b_ in range(BL):
                        nc.scalar.dma_start(
                            out=out_d[b_:b_ + 1,
                                      HARM_OFF[k_] + 512 * gstart_:
                                      HARM_OFF[k_] + 512 * (gstart_ + gs_)]
                            .rearrange("o (cc t) -> (o cc) t", t=512),
                            in_=hsb[b_:4 * gs_:4, :])
                pending.clear()

            def _emit_chunks(k, hooks=()):
                bs = BAND_SIZES[k]
                Bc = 512 // (bs // 32)
                nch = bs // 512
                lp, la = band_lhsT[k]
                pbt = ct[f'pb{k}']
                abt = ct[f'ab{k}']
                hooks = list(hooks)
                for gstart in range(0, nch, 8):
                    gs = min(8, nch - gstart)
                    hpt = ph.tile([128, 512], F32, tag="ph")
                    for cc in range(gs):
                        if cc == 2:
                            _flush_harm()
                        c_ = gstart + cc
                        ppt = pp.tile([128, 512], F32, tag="pp")
                        nc.tensor.matmul(out=ppt,
                                         lhsT=lp[0:4 * Bc, 128 * c_:128 * (c_ + 1)],
                                         rhs=pbt[0:4 * Bc, :],
                                         start=True, stop=False)
                        ntile = hot.tile([128, 512], BF16, tag="ntile")
                        nc.vector.tensor_scalar(out=ntile, in0=ppt, scalar1=MAGIC, scalar2=MAGIC,
                                                op0=ALU.add, op1=ALU.subtract)
                        nc.tensor.matmul(out=ppt, lhsT=ct['negI'], rhs=ntile,
                                         start=False, stop=True)
                        s = hot.tile([128, 512], F32, tag="sin_t")
                        nc.scalar.activation(out=s, in_=ppt, func=AF.Sin, scale=TWO_PI)
                        pat = pa.tile([128, 512], F32, tag="pa")
                        nc.tensor.matmul(out=pat,
                                         lhsT=la[0:3 * Bc, 128 * c_:128 * (c_ + 1)],
                                         rhs=abt[0:3 * Bc, :],
                                         start=True, stop=True)
                        prod = hot.tile([128, 512], BF16, tag="prod_t")
                        nc.vector.tensor_tensor(out=prod, in0=s, in1=pat, op=ALU.mult)

                        def _mk(hpt_=hpt, cc_=cc, prod_=prod, st=(cc == 0), sp=(cc == gs - 1)):
                            def _f():
                                nc.tensor.matmul(
                                    out=hpt_,
                                    lhsT=ct['selstrip'][:, 128 - 4 * cc_:256 - 4 * cc_],
                                    rhs=prod_, start=st, stop=sp, skip_group_check=True)
                            return _f
                        prev_osc = osc_q.pop(0) if osc_q else None
                        osc_q.append(_mk())
                        if prev_osc is not None:
                            prev_osc()
                    pending.append((hpt, gs, gstart, k))
                    if gs < 3:
                        _emit_osc()
                        _flush_harm()
                    if hooks:
                        hooks.pop(0)()

            # ---------------- emission
            for j in range(4):
                for k in (5, 4, 3, 2, 1, 0):
                    _stack_step(k, j)
            _stack_fin(5)
            _grids(5)
            Sr5 = _machinery_a(5, eng=nc.vector)
            _machinery_b(5, Sr5, eng=nc.vector)
            for k in (4, 3, 2, 1, 0):
                _stack_fin(k)
            for k in (4, 3, 2, 1, 0):
                _grids(k)
            for k in (5, 4, 3, 2, 1, 0):
                _noise_i(k)

            Sr_pend = {}

            def _hookA(k, extra=()):
                def f():
                    Sr_pend[k] = _machinery_a(k)
                    for g in extra:
                        g()
                return f

            def _hookB(k, nxt=None, extra=()):
                def f():
                    _machinery_b(k, Sr_pend[k])
                    if nxt is not None:
                        Sr_pend[nxt] = _machinery_a(nxt)
                    for g in extra:
                        g()
                return f

            def _nii():
                for k in (5, 4, 3, 2, 1, 0):
                    _noise_ii(k)

            _emit_chunks(5, hooks=[
                _hookA(4, extra=[_nii]),
                _hookB(4, 3, extra=[lambda: (_noise_iii(5), _noise_iii(4))]),
                _hookB(3, 2, extra=[lambda: (_noise_iii(3), _noise_iii(2))]),
                _hookB(2, 1, extra=[lambda: (_noise_iii(1), _noise_iii(0))])])
            _emit_chunks(4, hooks=[_hookB(1, 0), _hookB(0)])
            _emit_chunks(3)
            _emit_chunks(2)
            _emit_chunks(1)
            _emit_chunks(0)
            _emit_osc()
            _flush_harm()

    nc.finalize()
    return nc


def _prep_inputs(inputs):
    inp = {k: np.asarray(v, np.float32) for k, v in inputs.items()}
    shared = _build_shared(inp)
    in_maps = []
    for core in range(NCORE):
        m = dict(shared)
        sl = slice(core * BL, (core + 1) * BL)
        m['xT'] = np.concatenate([inp['x'][sl].T, np.ones((1, BL), np.float32)], axis=0)
        for k, bs in enumerate(BAND_SIZES):
            spf = bs // NNF
            m[f'noise{k}'] = np.ascontiguousarray(inp[f'noise_{k}'][sl].reshape(FR, spf),
                                                  dtype=np.float32)
        in_maps.append(m)
    return in_maps


def kernel(**inputs):
    if 'nc' not in _nc_cache:
        _nc_cache['nc'] = _build_nc()
    nc = _nc_cache['nc']
    in_maps = _prep_inputs(inputs)
    res = run_bass_kernel_spmd(nc, in_maps, list(range(NCORE)))
    out = np.concatenate([res.results[i]["out"] for i in range(NCORE)], axis=0)
    return out.astype(np.float32)


if __name__ == "__main__":
    import reference
    inp = reference.setup_inputs()
    out = kernel(**{k: np.asarray(v) for k, v in inp.items()})
    print("out", out.shape, out.dtype)


# revision 25
# speedup vs baseline: 1.0449x; 1.0449x over previous
"""Trainium2 Bass kernel for nn_Decoder (DDSP-style decoder).

Pure data-parallel over batch (32 -> 4 per core x 8 cores). Per core the
oscillator rows (4 batch x 32 osc) fill the 128 partitions exactly.
Phase is synthesized in *turns* by block-affine fp32 matmuls, range-reduced
with magic-constant rounding + an accumulating -I matmul, evaluated with the
ScalarE Sin LUT (accurate on [-pi, pi]), multiplied by the matmul-synthesized
amplitude envelope, and reduced over oscillators with a selector matmul.
The noise branch runs as real-DFT basis matmuls (no FFT instructions) at
float32r (full-rate PE).

Program structure is two "eras" split by a single activation-table switch:
era 1 = frontend + residual stacks + osc-grid activations (Sigmoid table),
era 2 = per-band scan machinery + noise branch + harmonic chunks (Sin table),
with band k's chunks interleaved right after band k's noise work.
"""
import numpy as np
import sys

sys.path.insert(0, "/opt/trn_rl_repo")

from concourse import bacc, mybir  # noqa: E402
from concourse.tile import TileContext  # noqa: E402
from concourse.bass_utils import run_bass_kernel_spmd  # noqa: E402

F32 = mybir.dt.float32
F32R = mybir.dt.float32r
BF16 = mybir.dt.bfloat16
ALU = mybir.AluOpType
BAND_SIZES = [512, 1024, 2048, 4096, 8192, 16384]
ADJUST = {512: 0.05, 1024: 0.03, 2048: 0.05, 4096: 0.25, 8192: 1.0, 16384: 20.0}
B, C, N_OSC, NNF = 32, 64, 32, 64
NCORE = 8
BL = B // NCORE          # 4 local batch
FR = BL * NNF            # 256 noise frames per core
MAGIC = float(1.5 * 2 ** 23)
TWO_PI = float(2 * np.pi)
TOTAL = 2 * sum(BAND_SIZES)  # 64512
USE_F32R_NOISE = True

_nc_cache = {}

W64_ORDER = ([f'up{i}d{dd}' for i in range(3) for dd in range(3)]
             + [f'find{dd}' for dd in range(3)]
             + [w for k in range(6) for w in
                [f't{k}0', f't{k}1', f't{k}2', f't{k}3', f'bf{k}', f'nup{k}']]
             + ['ident'])
W64_IDX = {n: i for i, n in enumerate(W64_ORDER)}
B64_ORDER = ([f'up{i}' for i in range(3)] + ['fin']
             + [b for k in range(6) for b in
                [f't{k}0', f't{k}1', f't{k}2', f't{k}3', f'bf{k}', f'nup{k}']])
B64_IDX = {n: i for i, n in enumerate(B64_ORDER)}
W3264_IDX = {}
for k in range(6):
    W3264_IDX[f'amp{k}'] = 2 * k
    W3264_IDX[f'frq{k}'] = 2 * k + 1

HARM_OFF = {}
NZ_OFF = {}
_off = 0
for _k, _bs in enumerate(BAND_SIZES):
    HARM_OFF[_k] = _off
    NZ_OFF[_k] = _off + _bs
    _off += 2 * _bs


# ---------------------------------------------------------------- host consts
def _interp_vecs(u):
    r = np.arange(u)
    f = (r + 0.5) / u - 0.5
    gm = np.where(r < u // 2, -f, 0.0)
    g0 = np.where(r < u // 2, 1 + f, 1 - f)
    gp = np.where(r >= u // 2, f, 0.0)
    return gm, g0, gp


def _build_U(n):
    eye = np.eye(n)
    spec = np.fft.rfft(eye, axis=-1)
    spec = np.pad(spec, ((0, 0), (0, n + 1 - spec.shape[-1])))
    return np.fft.irfft(spec, n=2 * n, axis=-1) * 2  # (n, 2n)


def _layout(ents):
    off = {}
    o = 0
    for name, r, cdim in ents:
        off[name] = (r, o, cdim)
        o += cdim
    return off, o


def _mega_ents():
    # f32 constants.  head0 | head1 | pb tail (split points for load priority)
    ents = [('wlin', C + 1, 4 * C), ('ubd4', BL * 4, BL * 8), ('ubd8', BL * 8, BL * 16),
            ('ubd16', BL * 16, BL * 32), ('bias64', C, len(B64_ORDER)),
            ('bias32', N_OSC, 12), ('ident128', 128, 128),
            ('w64', C, len(W64_ORDER) * C), ('w3264', C, 12 * N_OSC)]
    for k in (5, 0, 1, 2, 3, 4):
        ents.append((f'pb{k}', 128, 512))
    return ents


def _megar_ents():
    # float32r constants (noise branch DFT + coeff weights)
    ents = []
    for k in (5, 0, 1, 2, 3, 4):
        bs = BAND_SIZES[k]
        spf = bs // NNF
        nc_ = spf // 2 + 1
        ents.append((f'wc{k}', C + 1, nc_))
        if spf <= 128:
            ents.append((f'ft{k}', spf, spf))
            ents.append((f'ir{k}', spf, spf))
        else:
            for hh in range(2):
                ents.append((f'ft{k}_{hh}', 128, spf))
                ents.append((f'ir{k}_{hh}', 128, spf))
    return ents


def _megab_ents():
    # bf16 constants
    ents = [('negI', 128, 128), ('selstrip', 128, 256)]
    for k in (5, 0, 1, 2, 3, 4):
        ents.append((f'ab{k}', 128, 512))
    return ents


MEGA_OFF, MEGA_COLS = _layout(_mega_ents())
MEGAR_OFF, MEGAR_COLS = _layout(_megar_ents())
MEGAB_OFF, MEGAB_COLS = _layout(_megab_ents())
# load-priority split points in mega (cols)
MEGA_HEAD0 = MEGA_OFF['w64'][1]          # wlin/ubd/biases/ident128
MEGA_HEAD1 = MEGA_OFF['pb5'][1]          # w64/w3264
MEGA_HEAD2 = MEGA_OFF['pb0'][1]          # pb5
MEGAB_HEAD = MEGAB_OFF['ab0'][1]         # negI/selstrip/ab5
MEGAR_HEAD = MEGAR_OFF['wc0'][1]         # band-5 wc/ft/ir


def _band_bases(bs):
    u = bs // 32
    Bc = 512 // u
    gm, g0, gp = _interp_vecs(u)
    Gm, G0, Gp = np.cumsum(gm), np.cumsum(g0), np.cumsum(gp)
    pb = np.zeros((128, 512))
    ab = np.zeros((128, 512))
    inv = 1.0 / ADJUST[bs]
    sc = 8.0 / u   # compensates the u/8 folded into the incg grid scale
    for qq in range(Bc):
        cols = slice(qq * u, (qq + 1) * u)
        pb[0 * Bc + qq, cols] = Gm * sc
        pb[1 * Bc + qq, cols] = G0 * sc
        pb[2 * Bc + qq, cols] = Gp * sc
        pb[3 * Bc + qq, cols] = 1.0
        ab[0 * Bc + qq, cols] = gm * inv
        ab[1 * Bc + qq, cols] = g0 * inv
        ab[2 * Bc + qq, cols] = gp * inv
    return pb.astype(np.float32), ab.astype(np.float32)


def _band_fir(bs):
    spf = bs // NNF
    nc_ = spf // 2 + 1
    t = np.arange(spf)
    j_re = np.arange(nc_)
    j_im = np.arange(1, nc_ - 1)
    FT = np.concatenate([np.cos(2 * np.pi * np.outer(t, j_re) / spf),
                         -np.sin(2 * np.pi * np.outer(t, j_im) / spf)], axis=1)
    w = np.full(nc_, 2.0)
    w[0] = 1.0
    w[-1] = 1.0
    IR = np.concatenate([
        (w[:, None] * np.cos(2 * np.pi * np.outer(j_re, t) / spf)) / spf,
        (-2.0 * np.sin(2 * np.pi * np.outer(j_im, t) / spf)) / spf,
    ], axis=0) / ADJUST[bs]
    return FT.astype(np.float32), IR.astype(np.float32)


def _build_shared(inp):
    import ml_dtypes
    c = {}
    wl = np.zeros((4, C + 1, C), np.float32)
    for t in range(4):
        wl[t, :C] = inp['up_lin_w'][:, t::4]
        wl[t, C] = inp['up_lin_b'][t::4]
    c['wlin'] = wl.transpose(1, 0, 2).reshape(C + 1, 4 * C)   # (65, 256), block t
    for n in (4, 8, 16):
        U = _build_U(n)
        ub = np.zeros((BL * n, BL * 2 * n), np.float32)
        for b in range(BL):
            ub[b * n:(b + 1) * n, b * 2 * n:(b + 1) * 2 * n] = U
        c[f'ubd{n}'] = ub

    w64 = np.zeros((C, len(W64_ORDER) * C), np.float32)

    def put64(name, m):
        i = W64_IDX[name]
        w64[:, i * C:(i + 1) * C] = m

    for i in range(3):
        for dd in range(3):
            put64(f'up{i}d{dd}', inp['up_conv_w'][i, :, :, dd].T)
    for dd in range(3):
        put64(f'find{dd}', inp['up_final_w'][:, :, dd].T)
    for k in range(6):
        for j in range(4):
            put64(f't{k}{j}', inp['t_w'][k, j].T)
        put64(f'bf{k}', inp['band_final_w'][k].T)
        put64(f'nup{k}', inp['noise_up_w'][k].T)
    put64('ident', np.eye(C))
    c['w64'] = w64

    w32 = np.zeros((C, 12 * N_OSC), np.float32)
    for k in range(6):
        w32[:, W3264_IDX[f'amp{k}'] * N_OSC:(W3264_IDX[f'amp{k}'] + 1) * N_OSC] = inp['osc_amp_w'][k].T
        w32[:, W3264_IDX[f'frq{k}'] * N_OSC:(W3264_IDX[f'frq{k}'] + 1) * N_OSC] = inp['osc_freq_w'][k].T
    c['w3264'] = w32

    b64 = np.zeros((C, len(B64_ORDER)), np.float32)
    for i in range(3):
        b64[:, B64_IDX[f'up{i}']] = inp['up_conv_b'][i]
    b64[:, B64_IDX['fin']] = inp['up_final_b']
    for k in range(6):
        for j in range(4):
            b64[:, B64_IDX[f't{k}{j}']] = inp['t_b'][k, j]
        b64[:, B64_IDX[f'bf{k}']] = inp['band_final_b'][k]
        b64[:, B64_IDX[f'nup{k}']] = inp['noise_up_b'][k]
    c['bias64'] = b64

    b32 = np.zeros((N_OSC, 12), np.float32)
    for k in range(6):
        b32[:, W3264_IDX[f'amp{k}']] = inp['osc_amp_b'][k]
        b32[:, W3264_IDX[f'frq{k}']] = inp['osc_freq_b'][k]
    c['bias32'] = b32
    c['ident128'] = np.eye(128, dtype=np.float32)

    cr = {}
    cb = {'negI': (-np.eye(128)).astype(np.float32)}
    sel = np.zeros((128, 256), np.float32)
    for b in range(BL):
        sel[b * N_OSC:(b + 1) * N_OSC, 128 + b] = 1.0
    cb['selstrip'] = sel

    for k, bs in enumerate(BAND_SIZES):
        nc_ = bs // NNF // 2 + 1
        w = np.zeros((C + 1, nc_), np.float32)
        w[:C] = inp[f'noise_coeff_w_{k}'].T
        w[C] = inp[f'noise_coeff_b_{k}']
        if k == 0:
            w[:, 1:] = 0.0
        cr[f'wc{k}'] = w
        FT, IR = _band_fir(bs)
        if bs // NNF <= 128:
            cr[f'ft{k}'] = FT
            cr[f'ir{k}'] = IR
        else:
            cr[f'ft{k}_0'], cr[f'ft{k}_1'] = FT[0:128], FT[128:256]
            cr[f'ir{k}_0'], cr[f'ir{k}_1'] = IR[0:128], IR[128:256]
        pb, ab = _band_bases(bs)
        c[f'pb{k}'] = pb
        cb[f'ab{k}'] = ab

    mega = np.zeros((128, MEGA_COLS), np.float32)
    for name, (r, o, cd) in MEGA_OFF.items():
        mega[0:r, o:o + cd] = c[name]
    megar = np.zeros((128, MEGAR_COLS), np.float32)
    for name, (r, o, cd) in MEGAR_OFF.items():
        megar[0:r, o:o + cd] = cr[name]
    megab = np.zeros((128, MEGAB_COLS), np.float32)
    for name, (r, o, cd) in MEGAB_OFF.items():
        megab[0:r, o:o + cd] = cb[name]
    return {'mega': mega, 'megar': megar,
            'megab': megab.astype(ml_dtypes.bfloat16)}


# ---------------------------------------------------------------- bass build
def _build_nc():
    nc = bacc.Bacc('TRN2', num_devices=NCORE)
    AF = mybir.ActivationFunctionType

    d = {}
    d['xT'] = nc.dram_tensor("xT", [C + 1, BL], F32, kind="ExternalInput")
    d['mega'] = nc.dram_tensor("mega", [128, MEGA_COLS], F32, kind="ExternalInput")
    d['megar'] = nc.dram_tensor("megar", [128, MEGAR_COLS], F32R if USE_F32R_NOISE else F32, kind="ExternalInput")
    d['megab'] = nc.dram_tensor("megab", [128, MEGAB_COLS], BF16, kind="ExternalInput")
    for k, bs in enumerate(BAND_SIZES):
        spf = bs // NNF
        d[f'noise{k}'] = nc.dram_tensor(f"noise{k}", [FR, spf], F32, kind="ExternalInput")
    out_d = nc.dram_tensor("out", [BL, TOTAL], F32, kind="ExternalOutput")

    with TileContext(nc) as tc:
        with tc.tile_pool(name="const", bufs=1) as cp, \
             tc.tile_pool(name="work", bufs=1) as wp, \
             tc.tile_pool(name="hot", bufs=4) as hot, \
             tc.tile_pool(name="dram", bufs=1, space="DRAM") as dp, \
             tc.tile_pool(name="pp", bufs=3, space="PSUM") as pp, \
             tc.tile_pool(name="pa", bufs=2, space="PSUM") as pa, \
             tc.tile_pool(name="ph", bufs=3, space="PSUM") as ph:

            # ---------------- const loads (priority order; chunks-of-band-5
            # need pb5/ab5/negI/selstrip early, then band-5 noise consts)
            mega = cp.tile([128, MEGA_COLS], F32, tag="mega")
            nc.sync.dma_start(out=mega[:, 0:MEGA_HEAD0],
                              in_=d['mega'][:, 0:MEGA_HEAD0])
            xT = cp.tile([C + 1, BL], F32, tag="xT")
            nc.sync.dma_start(out=xT, in_=d['xT'][:, :])
            nc.sync.dma_start(out=mega[:, MEGA_HEAD0:MEGA_HEAD1],
                              in_=d['mega'][:, MEGA_HEAD0:MEGA_HEAD1])
            nc.sync.dma_start(out=mega[:, MEGA_HEAD1:MEGA_HEAD2],
                              in_=d['mega'][:, MEGA_HEAD1:MEGA_HEAD2])
            megab = cp.tile([128, MEGAB_COLS], BF16, tag="megab")
            nc.sync.dma_start(out=megab[:, 0:MEGAB_HEAD],
                              in_=d['megab'][:, 0:MEGAB_HEAD])
            megar = cp.tile([128, MEGAR_COLS], F32R if USE_F32R_NOISE else F32, tag="megar")
            nc.sync.dma_start(out=megar[:, 0:MEGAR_HEAD],
                              in_=d['megar'][:, 0:MEGAR_HEAD])
            nbt = {}
            for k in (5, 0, 1, 2, 3, 4):
                spf = BAND_SIZES[k] // NNF
                nb0 = cp.tile([128, spf], F32, tag=f"nb0_{k}")
                nb1 = cp.tile([128, spf], F32, tag=f"nb1_{k}")
                nc.sync.dma_start(out=nb0, in_=d[f'noise{k}'][0:128, :])
                nc.sync.dma_start(out=nb1, in_=d[f'noise{k}'][128:256, :])
                nbt[k] = (nb0, nb1)
            nc.sync.dma_start(out=megar[:, MEGAR_HEAD:MEGAR_COLS],
                              in_=d['megar'][:, MEGAR_HEAD:MEGAR_COLS])
            nc.sync.dma_start(out=megab[:, MEGAB_HEAD:MEGAB_COLS],
                              in_=d['megab'][:, MEGAB_HEAD:MEGAB_COLS])
            nc.sync.dma_start(out=mega[:, MEGA_HEAD2:MEGA_COLS],
                              in_=d['mega'][:, MEGA_HEAD2:MEGA_COLS])

            ct = {}
            for name, (r, o, cd) in MEGA_OFF.items():
                ct[name] = mega[0:r, o:o + cd]
            for name, (r, o, cd) in MEGAR_OFF.items():
                ct[name] = megar[0:r, o:o + cd]
            for name, (r, o, cd) in MEGAB_OFF.items():
                ct[name] = megab[0:r, o:o + cd]

            def w64s(name):
                i = W64_IDX[name]
                return ct['w64'][:, i * C:(i + 1) * C]

            def w32s(name):
                i = W3264_IDX[name]
                return ct['w3264'][:, i * N_OSC:(i + 1) * N_OSC]

            def b64s(name):
                return ct['bias64'][:, B64_IDX[name]:B64_IDX[name] + 1]

            def b32s(name):
                i = W3264_IDX[name]
                return ct['bias32'][:, i:i + 1]

            ident64 = w64s('ident')

            # ================ ERA 1: frontend + stacks + grid activations
            h = wp.tile([C, 16], F32, tag="h0")
            for t in range(4):
                pt = pp.tile([C, BL], F32, tag="pp")
                nc.tensor.matmul(out=pt, lhsT=ct['wlin'][:, t * C:(t + 1) * C], rhs=xT,
                                 start=True, stop=True)
                nc.vector.tensor_copy(out=h.rearrange("c (b t) -> c b t", t=4)[:, :, t], in_=pt)

            for i, n in enumerate((4, 8, 16)):
                pt1 = pp.tile([BL * n, C], F32, tag="pp")
                nc.tensor.transpose(out=pt1, in_=h, identity=ct['ident128'][0:C, 0:C])
                t1 = wp.tile([BL * n, C], F32, tag=f"fe_t1_{i}")
                nc.vector.tensor_copy(out=t1, in_=pt1)
                pt2 = pp.tile([BL * 2 * n, C], F32, tag="pp")
                nc.tensor.matmul(out=pt2, lhsT=ct[f'ubd{n}'], rhs=t1, start=True, stop=True)
                t2 = wp.tile([BL * 2 * n, C], F32, tag=f"fe_t2_{i}")
                nc.vector.tensor_copy(out=t2, in_=pt2)
                pt3 = pp.tile([C, BL * 2 * n], F32, tag="pp")
                nc.tensor.transpose(out=pt3, in_=t2,
                                    identity=ct['ident128'][0:BL * 2 * n, 0:BL * 2 * n])
                hu = wp.tile([C, BL * 2 * n], F32, tag=f"fe_hu_{i}")
                nc.vector.tensor_copy(out=hu, in_=pt3)
                m = 2 * n
                hu3 = hu.rearrange("c (b t) -> c b t", b=BL)
                pc = pp.tile([C, BL, m], F32, tag="pp")
                nc.tensor.matmul(out=pc[:, :, :], lhsT=w64s(f'up{i}d1'), rhs=hu3[:, :, :],
                                 start=True, stop=False)
                nc.tensor.matmul(out=pc[:, :, 1:m], lhsT=w64s(f'up{i}d0'), rhs=hu3[:, :, 0:m - 1],
                                 start=False, stop=False)
                nc.tensor.matmul(out=pc[:, :, 0:m - 1], lhsT=w64s(f'up{i}d2'), rhs=hu3[:, :, 1:m],
                                 start=False, stop=True)
                h = wp.tile([C, BL * m], F32, tag=f"fe_h_{i}")
                nc.scalar.activation(out=h.rearrange("c (b t) -> c b t", b=BL), in_=pc,
                                     func=AF.Prelu, bias=b64s(f'up{i}'), scale=1.0, alpha=0.2)

            h3 = h.rearrange("c (b t) -> c b t", b=BL)
            pf = pp.tile([C, BL, 32], F32, tag="pp")
            nc.tensor.matmul(out=pf[:, :, :], lhsT=w64s('find1'), rhs=h3[:, :, :],
                             start=True, stop=False)
            nc.tensor.matmul(out=pf[:, :, 1:32], lhsT=w64s('find0'), rhs=h3[:, :, 0:31],
                             start=False, stop=False)
            nc.tensor.matmul(out=pf[:, :, 0:31], lhsT=w64s('find2'), rhs=h3[:, :, 1:32],
                             start=False, stop=True)
            hfin = cp.tile([C, 128], F32, tag="hfin")
            nc.scalar.activation(out=hfin.rearrange("c (b t) -> c b t", b=BL), in_=pf,
                                 func=AF.Identity, bias=b64s('fin'), scale=1.0)

            # Emission schedule (band 5 owns half the chunk work, so it
            # goes first and its 48us chunk block starts ~20us in; the other
            # bands' grid machinery + gather chains ride in hooks between
            # chunk groups, on engines the chunk pipeline leaves idle):
            #   frontend -> stacks+grids (5..0) -> machinery5+chain5
            #   -> noise (3 pipelined passes, all bands)
            #   -> chunks5 [hooks: A4|B4,A3|B3,A2|B2,A1] -> chunks4 [B1,A0|B0]
            #   -> chunks3..0
            GC = {'ipr': 0, 'incg': 128, 'inx': 256, 'Cf': 384,
                  'apr': 512, 'ampg': 640, 'anx': 768}
            zt32 = cp.tile([N_OSC, 32], F32, tag="zt32")
            nc.vector.memset(zt32, 0.0)
            ones_f32 = wp.tile([1, FR], F32, tag="ones_f32")
            nc.vector.memset(ones_f32, 1.0)
            # scan-reset mask: state = mask*state + Sr resets at batch starts
            maskt = cp.tile([N_OSC, 128], F32, tag="maskt")
            nc.vector.memset(maskt, 1.0)
            for b in range(BL):
                nc.vector.memset(maskt[:, 32 * b:32 * b + 1], 0.0)

            zfs, Gs, siggs = {}, {}, {}
            band_lhsT = {}
            pending = []
            osc_q = []

            zcur = {}

            def _stack_step(k, j):
                # one residual link; j-major emission keeps ScalarE streaming
                z = zcur.get(k, hfin)
                pz = pp.tile([C, 128], F32, tag="pp")
                nc.tensor.matmul(out=pz, lhsT=w64s(f't{k}{j}'), rhs=z, start=True, stop=False)
                nc.tensor.matmul(out=pz, lhsT=ident64, rhs=z, start=False, stop=True)
                z = wp.tile([C, 128], F32, tag=f"z_{k}_{j % 2}")
                nc.scalar.activation(out=z, in_=pz, func=AF.Prelu,
                                     bias=b64s(f't{k}{j}'), scale=1.0, alpha=0.2)
                zcur[k] = z

            def _stack_fin(k):
                pz = pp.tile([C, 128], F32, tag="pp")
                nc.tensor.matmul(out=pz, lhsT=w64s(f'bf{k}'), rhs=zcur[k], start=True, stop=True)
                zf = cp.tile([C, 128], F32, tag=f"zf_{k}")
                nc.scalar.activation(out=zf, in_=pz, func=AF.Identity, bias=b64s(f'bf{k}'),
                                     scale=1.0)
                zfs[k] = zf

            def _grids(k):
                zf = zfs[k]
                G = cp.tile([N_OSC, 896], F32, tag=f"G_{k}")
                Gs[k] = G
                pg = pa.tile([N_OSC, 128], F32, tag="pa")
                nc.tensor.matmul(out=pg, lhsT=w32s(f'amp{k}'), rhs=zf, start=True, stop=True)
                nc.scalar.activation(out=G[:, GC['ampg']:GC['ampg'] + 128], in_=pg,
                                     func=AF.Abs, bias=b32s(f'amp{k}'), scale=1.0)
                pg2 = pa.tile([N_OSC, 128], F32, tag="pa")
                nc.tensor.matmul(out=pg2, lhsT=w32s(f'frq{k}'), rhs=zf, start=True, stop=True)
                sigg = wp.tile([N_OSC, 128], F32, tag=f"sigg_{k}")
                nc.scalar.activation(out=sigg, in_=pg2, func=AF.Sigmoid, bias=b32s(f'frq{k}'),
                                     scale=1.0)
                siggs[k] = sigg

            def _machinery_a(k, eng=None):
                # incg2 = (freq/2)*(u/8) folded scale (pb G-rows host-scaled
                # by 8/u).  eng=nc.gpsimd rides in chunk-phase hooks (Pool is
                # idle there); band 5 runs pre-chunk on DVE only, avoiding
                # cross-engine sem hops on its critical path.
                eng = eng or nc.gpsimd
                bs = BAND_SIZES[k]
                u = bs // 32
                lf = 0.05 if bs == 512 else 0.01
                G = Gs[k]
                sigg = siggs[k]
                incg = G[:, GC['incg']:GC['incg'] + 128]
                nc.vector.tensor_scalar(out=incg, in0=sigg,
                                        scalar1=float((1.0 - lf) / 2.0 * u / 8.0),
                                        scalar2=float(lf / 2.0 * u / 8.0),
                                        op0=ALU.mult, op1=ALU.add)

                def shifted(src, pr, nx):
                    s3 = src.rearrange("o (b q) -> o b q", b=BL)
                    p3 = pr.rearrange("o (b q) -> o b q", b=BL)
                    n3 = nx.rearrange("o (b q) -> o b q", b=BL)
                    eng.tensor_copy(out=p3[:, :, 1:32], in_=s3[:, :, 0:31])
                    eng.tensor_copy(out=p3[:, :, 0:1], in_=s3[:, :, 0:1])
                    eng.tensor_copy(out=n3[:, :, 0:31], in_=s3[:, :, 1:32])
                    eng.tensor_copy(out=n3[:, :, 31:32], in_=s3[:, :, 31:32])

                ipr = G[:, GC['ipr']:GC['ipr'] + 128]
                inx = G[:, GC['inx']:GC['inx'] + 128]
                ampg = G[:, GC['ampg']:GC['ampg'] + 128]
                apr = G[:, GC['apr']:GC['apr'] + 128]
                anx = G[:, GC['anx']:GC['anx'] + 128]
                shifted(incg, ipr, inx)
                shifted(ampg, apr, anx)
                # S = ipr2 + 6*incg2 + inx2  (per-block increment, turns)
                t1 = wp.tile([N_OSC, 128], F32, tag=f"sg1_{k % 2}")
                eng.tensor_scalar(out=t1, in0=incg, scalar1=6.0, scalar2=None,
                                        op0=ALU.mult)
                t2 = wp.tile([N_OSC, 128], F32, tag=f"sg2_{k % 2}")
                eng.tensor_tensor(out=t2, in0=t1, in1=ipr, op=ALU.add)
                S = wp.tile([N_OSC, 128], F32, tag=f"sgS_{k % 2}")
                eng.tensor_tensor(out=S, in0=t2, in1=inx, op=ALU.add)
                Sn = wp.tile([N_OSC, 128], F32, tag=f"sgSn_{k % 2}")
                eng.tensor_scalar(out=Sn, in0=S, scalar1=MAGIC, scalar2=MAGIC,
                                        op0=ALU.add, op1=ALU.subtract)
                Sr = wp.tile([N_OSC, 128], F32, tag=f"sgSr_{k % 2}")
                eng.tensor_tensor(out=Sr, in0=S, in1=Sn, op=ALU.subtract)
                return Sr

            def _machinery_b(k, Sr, eng=None):
                eng = eng or nc.gpsimd
                bs = BAND_SIZES[k]
                Bc = 512 // (bs // 32)
                nch = bs // 512
                G = Gs[k]
                Pt = wp.tile([N_OSC, 128], F32, tag=f"sgPt_{k % 2}")
                nc.vector.tensor_tensor_scan(out=Pt, data0=maskt, data1=Sr,
                                             initial=0.0, op0=ALU.mult, op1=ALU.add)
                Ce = wp.tile([N_OSC, 128], F32, tag=f"sgCe_{k % 2}")
                Ce3 = Ce.rearrange("o (b q) -> o b q", b=BL)
                eng.tensor_copy(out=Ce3[:, :, 1:32],
                                in_=Pt.rearrange("o (b q) -> o b q", b=BL)[:, :, 0:31])
                eng.memset(Ce3[:, :, 0:1], 0.0)
                Cn = wp.tile([N_OSC, 128], F32, tag=f"sgCn_{k % 2}")
                eng.tensor_scalar(out=Cn, in0=Ce, scalar1=MAGIC, scalar2=MAGIC,
                                        op0=ALU.add, op1=ALU.subtract)
                eng.tensor_tensor(out=G[:, GC['Cf']:GC['Cf'] + 128],
                                  in0=Ce, in1=Cn, op=ALU.subtract)

                Tall = wp.tile([N_OSC, 896], F32, tag=f"Tall_{k}")
                nc.vector.transpose(out=Tall, in_=G)
                # DMA chain realizing the 4-dim permute (c,q,g,m)->(g,q,c,m).
                # SBUF partition dims cannot be split in DMA APs, so the
                # (c q)->(q c) row transpose happens DRAM->DRAM; when nch==1
                # or Bc==1 no row transpose is needed and Tall goes straight
                # to scrB (partition dim stays whole, cols split by g).
                lp = cp.tile([128, nch * 128], F32, tag=f"lp_{k}")
                la = cp.tile([128, nch * 128], BF16, tag=f"la_{k}")
                if Bc == 1:
                    # one hop: rows (r, g), SBUF partition dim stays outermost
                    scrB2 = dp.tile([7 * N_OSC, 128], F32, tag=f"scrB_{k}")
                    nc.sync.dma_start(
                        out=scrB2.rearrange("(r g) m -> r g m", g=7),
                        in_=Tall.rearrange("r (g m) -> r g m", m=128))
                    src_p = scrB2.rearrange("(r g) m -> g r m", g=7)
                    nc.sync.dma_start(
                        out=lp[0:4, :].rearrange("p (c m) -> p c m", m=128),
                        in_=src_p[0:4, :, :])
                    nc.gpsimd.dma_start(
                        out=la[0:3, :].rearrange("p (c m) -> p c m", m=128),
                        in_=src_p[4:7, :, :])
                else:
                    scrA = dp.tile([N_OSC, 896], F32, tag=f"scrA_{k}")
                    nc.sync.dma_start(out=scrA, in_=Tall)
                    if nch == 1:
                        scrT = scrA
                    else:
                        scrT = dp.tile([N_OSC, 896], F32, tag=f"scrT_{k}")
                        nc.sync.dma_start(
                            out=scrT.rearrange("(q c) m -> q c m", c=nch),
                            in_=scrA.rearrange("(c q) m -> q c m", q=Bc))
                    scrB = dp.tile([7 * N_OSC, 128], F32, tag=f"scrB_{k}")
                    nc.sync.dma_start(
                        out=scrB.rearrange("(g r) m -> g r m", g=7),
                        in_=scrT.rearrange("r (g m) -> g r m", m=128))
                    nc.sync.dma_start(
                        out=lp[0:4 * Bc, :].rearrange("p (c m) -> p c m", m=128),
                        in_=scrB[0:4 * N_OSC, :].rearrange("(p c) m -> p c m", c=nch))
                    nc.gpsimd.dma_start(
                        out=la[0:3 * Bc, :].rearrange("p (c m) -> p c m", m=128),
                        in_=scrB[4 * N_OSC:7 * N_OSC, :].rearrange("(p c) m -> p c m", c=nch))
                band_lhsT[k] = (lp, la)

            # noise branch in 3 pipelined passes (per-band tiles so bands
            # overlap; each pass's deps are ready when the queue reaches it)
            NDT = F32R if USE_F32R_NOISE else F32
            nTs_all, naug_all, pc_all, chat_all = {}, {}, {}, {}

            def _noise_i(k):
                spf = BAND_SIZES[k] // NNF
                zf = zfs[k]
                zf3 = zf.rearrange("c (b t) -> c b t", b=BL)
                zrep = zf3.unsqueeze(-1).broadcast_to([C, BL, 32, 2])
                pn = pp.tile([C, FR], F32, tag="pp")
                nc.tensor.matmul(out=pn, lhsT=w64s(f'nup{k}'), rhs=zrep, start=True, stop=True)
                naug = cp.tile([C + 1, FR], NDT, tag=f"naug_{k}")
                nc.vector.tensor_copy(out=naug[C:C + 1, :], in_=ones_f32)
                nc.scalar.activation(out=naug[0:C, :], in_=pn, func=AF.Prelu,
                                     bias=b64s(f'nup{k}'), scale=1.0, alpha=0.2)
                naug_all[k] = naug
                nb0, nb1 = nbt[k]
                if spf <= 128:
                    nT = wp.tile([spf, FR], NDT, tag=f"nT_{k}")
                    for hh, nb in enumerate((nb0, nb1)):
                        ptr = pp.tile([spf, 128], F32, tag="pp")
                        nc.tensor.transpose(out=ptr, in_=nb, identity=ct['ident128'])
                        nc.vector.tensor_copy(out=nT[:, 128 * hh:128 * (hh + 1)], in_=ptr)
                    nTs_all[k] = [nT]
                else:
                    nT0 = wp.tile([128, FR], NDT, tag="nT5_0")
                    nT1 = wp.tile([128, FR], NDT, tag="nT5_1")
                    for hh, nb in enumerate((nb0, nb1)):
                        for half, dst in ((0, nT0), (1, nT1)):
                            ptr = pp.tile([128, 128], F32, tag="pp")
                            nc.tensor.transpose(out=ptr, in_=nb[:, 128 * half:128 * (half + 1)],
                                                identity=ct['ident128'])
                            nc.vector.tensor_copy(out=dst[:, 128 * hh:128 * (hh + 1)], in_=ptr)
                    nTs_all[k] = [nT0, nT1]

            def _noise_ii(k):
                spf = BAND_SIZES[k] // NNF
                nc_ = spf // 2 + 1
                naug = naug_all[k]
                if nc_ <= 128:
                    pcA = pa.tile([nc_, FR], F32, tag="pa")
                    nc.tensor.matmul(out=pcA, lhsT=ct[f'wc{k}'], rhs=naug, start=True, stop=True)
                    pcB = None
                else:
                    pcA = pa.tile([128, FR], F32, tag="pa")
                    nc.tensor.matmul(out=pcA, lhsT=ct[f'wc{k}'][:, 0:128], rhs=naug,
                                     start=True, stop=True)
                    pcB = pp.tile([1, FR], F32, tag="pp")
                    nc.tensor.matmul(out=pcB, lhsT=ct[f'wc{k}'][:, 128:nc_], rhs=naug,
                                     start=True, stop=True)
                if spf <= 128:
                    chat = wp.tile([spf, FR], F32, tag=f"chat_{k}")
                    nc.scalar.copy(out=chat[0:nc_, :], in_=pcA)
                    if spf > nc_:
                        nc.scalar.dma_start(out=chat[nc_:spf, :], in_=chat[1:nc_ - 1, :])
                    chat_all[k] = [chat]
                else:
                    ch0 = wp.tile([128, FR], F32, tag="chat5_0")
                    nc.scalar.copy(out=ch0, in_=pcA)
                    ch1 = wp.tile([128, FR], F32, tag="chat5_1")
                    nc.scalar.copy(out=ch1, in_=pcA)
                    nc.scalar.copy(out=ch1[0:1, :], in_=pcB)
                    chat_all[k] = [ch0, ch1]

            def _noise_iii(k):
                bs = BAND_SIZES[k]
                spf = bs // NNF
                nTs = nTs_all[k]
                chats = chat_all[k]
                if spf <= 128:
                    psp = pp.tile([spf, FR], F32, tag="pp")
                    nc.tensor.matmul(out=psp, lhsT=ct[f'ft{k}'], rhs=nTs[0], start=True, stop=True)
                    sA = wp.tile([spf, FR], NDT, tag=f"sA_{k}")
                    nc.vector.tensor_tensor(out=sA, in0=chats[0], in1=psp, op=ALU.mult)
                    sAs = [sA]
                else:
                    sAs = []
                    for half in range(2):
                        psp = pp.tile([128, FR], F32, tag="pp")
                        nc.tensor.matmul(out=psp, lhsT=ct['ft5_0'][:, 128 * half:128 * (half + 1)],
                                         rhs=nTs[0], start=True, stop=False)
                        nc.tensor.matmul(out=psp, lhsT=ct['ft5_1'][:, 128 * half:128 * (half + 1)],
                                         rhs=nTs[1], start=False, stop=True)
                        sA = wp.tile([128, FR], NDT, tag=f"sA5_{half}")
                        nc.vector.tensor_tensor(out=sA, in0=chats[half], in1=psp, op=ALU.mult)
                        sAs.append(sA)
                for fg in range(2):
                    pnz = ph.tile([128, spf], F32, tag="ph")
                    if spf <= 128:
                        nc.tensor.matmul(out=pnz, lhsT=sAs[0][:, 128 * fg:128 * (fg + 1)],
                                         rhs=ct[f'ir{k}'], start=True, stop=True)
                    else:
                        nc.tensor.matmul(out=pnz, lhsT=sAs[0][:, 128 * fg:128 * (fg + 1)],
                                         rhs=ct['ir5_0'], start=True, stop=False)
                        nc.tensor.matmul(out=pnz, lhsT=sAs[1][:, 128 * fg:128 * (fg + 1)],
                                         rhs=ct['ir5_1'], start=False, stop=True)
                    nzs = wp.tile([128, spf], F32, tag=f"nzs_{k}")
                    nc.vector.tensor_copy(out=nzs, in_=pnz)
                    for j in range(2):
                        b_ = 2 * fg + j
                        nc.sync.dma_start(
                            out=out_d[b_:b_ + 1, NZ_OFF[k]:NZ_OFF[k] + bs]
                            .rearrange("o (f t) -> (o f) t", t=spf),
                            in_=nzs[NNF * j:NNF * (j + 1), :])

            def _emit_osc():
                while osc_q:
                    osc_q.pop(0)()

            def _flush_harm():
                for (hpt_, gs_, gstart_, k_) in pending:
                    hsb = wp.tile([4 * gs_, 512], F32, tag="hsb")
                    nc.scalar.copy(out=hsb, in_=hpt_[0:4 * gs_, :])
                    for b_ in range(BL):
                        nc.scalar.dma_start(
                            out=out_d[b_:b_ + 1,
                                      HARM_OFF[k_] + 512 * gstart_:
                                      HARM_OFF[k_] + 512 * (gstart_ + gs_)]
                            .rearrange("o (cc t) -> (o cc) t", t=512),
                            in_=hsb[b_:4 * gs_:4, :])
                pending.clear()

            def _emit_chunks(k, hooks=()):
                bs = BAND_SIZES[k]
                Bc = 512 // (bs // 32)
                nch = bs // 512
                lp, la = band_lhsT[k]
                pbt = ct[f'pb{k}']
                abt = ct[f'ab{k}']
                hooks = list(hooks)
                for gstart in range(0, nch, 8):
                    gs = min(8, nch - gstart)
                    hpt = ph.tile([128, 512], F32, tag="ph")
                    for cc in range(gs):
                        if cc == 2:
                            _flush_harm()
                        c_ = gstart + cc
                        ppt = pp.tile([128, 512], F32, tag="pp")
                        nc.tensor.matmul(out=ppt,
                                         lhsT=lp[0:4 * Bc, 128 * c_:128 * (c_ + 1)],
                                         rhs=pbt[0:4 * Bc, :],
                                         start=True, stop=False)
                        ntile = hot.tile([128, 512], BF16, tag="ntile")
                        nc.vector.tensor_scalar(out=ntile, in0=ppt, scalar1=MAGIC, scalar2=MAGIC,
                                                op0=ALU.add, op1=ALU.subtract)
                        nc.tensor.matmul(out=ppt, lhsT=ct['negI'], rhs=ntile,
                                         start=False, stop=True)
                        s = hot.tile([128, 512], F32, tag="sin_t")
                        nc.scalar.activation(out=s, in_=ppt, func=AF.Sin, scale=TWO_PI)
                        pat = pa.tile([128, 512], F32, tag="pa")
                        nc.tensor.matmul(out=pat,
                                         lhsT=la[0:3 * Bc, 128 * c_:128 * (c_ + 1)],
                                         rhs=abt[0:3 * Bc, :],
                                         start=True, stop=True)
                        prod = hot.tile([128, 512], BF16, tag="prod_t")
                        nc.vector.tensor_tensor(out=prod, in0=s, in1=pat, op=ALU.mult)

                        def _mk(hpt_=hpt, cc_=cc, prod_=prod, st=(cc == 0), sp=(cc == gs - 1)):
                            def _f():
                                nc.tensor.matmul(
                                    out=hpt_,
                                    lhsT=ct['selstrip'][:, 128 - 4 * cc_:256 - 4 * cc_],
                                    rhs=prod_, start=st, stop=sp, skip_group_check=True)
                            return _f
                        prev_osc = osc_q.pop(0) if osc_q else None
                        osc_q.append(_mk())
                        if prev_osc is not None:
                            prev_osc()
                    pending.append((hpt, gs, gstart, k))
                    if gs < 3:
                        _emit_osc()
                        _flush_harm()
                    if hooks:
                        hooks.pop(0)()

            # ---------------- emission
            for j in range(4):
                for k in (5, 4, 3, 2, 1, 0):
                    _stack_step(k, j)
            _stack_fin(5)
            _grids(5)
            Sr5 = _machinery_a(5, eng=nc.vector)
            _machinery_b(5, Sr5, eng=nc.vector)
            for k in (4, 3, 2, 1, 0):
                _stack_fin(k)
            for k in (4, 3, 2, 1, 0):
                _grids(k)
            for k in (5, 4, 3, 2, 1, 0):
                _noise_i(k)

            for k in (5, 4, 3, 2, 1, 0):
                _noise_ii(k)
            for k in (5, 4, 3, 2, 1, 0):
                _noise_iii(k)

            Sr_pend = {}

            def _hookA(k, extra=()):
                def f():
                    Sr_pend[k] = _machinery_a(k)
                    for g in extra:
                        g()
                return f

            def _hookB(k, nxt=None, extra=()):
                def f():
                    _machinery_b(k, Sr_pend[k])
                    if nxt is not None:
                        Sr_pend[nxt] = _machinery_a(nxt)
                    for g in extra:
                        g()
                return f

            _emit_chunks(5, hooks=[
                _hookA(4),
                _hookB(4, 3),
                _hookB(3, 2),
                _hookB(2, 1)])
            _emit_chunks(4, hooks=[_hookB(1, 0), _hookB(0)])
            _emit_chunks(3)
            _emit_chunks(2)
            _emit_chunks(1)
            _emit_chunks(0)
            _emit_osc()
            _flush_harm()

    nc.finalize()
    return nc


def _prep_inputs(inputs):
    inp = {k: np.asarray(v, np.float32) for k, v in inputs.items()}
    shared = _build_shared(inp)
    in_maps = []
    for core in range(NCORE):
        m = dict(shared)
        sl = slice(core * BL, (core + 1) * BL)
        m['xT'] = np.concatenate([inp['x'][sl].T, np.ones((1, BL), np.float32)], axis=0)
        for k, bs in enumerate(BAND_SIZES):
            spf = bs // NNF
            m[f'noise{k}'] = np.ascontiguousarray(inp[f'noise_{k}'][sl].reshape(FR, spf),
                                                  dtype=np.float32)
        in_maps.append(m)
    return in_maps


def kernel(**inputs):
    if 'nc' not in _nc_cache:
        _nc_cache['nc'] = _build_nc()
    nc = _nc_cache['nc']
    in_maps = _prep_inputs(inputs)
    res = run_bass_kernel_spmd(nc, in_maps, list(range(NCORE)))
    out = np.concatenate([res.results[i]["out"] for i in range(NCORE)], axis=0)
    return out.astype(np.float32)


if __name__ == "__main__":
    import reference
    inp = reference.setup_inputs()
    out = kernel(**{k: np.asarray(v) for k, v in inp.items()})
    print("out", out.shape, out.dtype)
